# revision 1
# baseline (speedup 1.0000x reference)
"""Trainium2 Bass kernel v2 for nn_LstmModel: B=512, T=256, H=512 LSTM + FC head.

DP-8 (64 batch/core) split into 2 interleaved waves of 32 batch rows each, so
one wave's elementwise tail (ACT/DVE/Pool) hides under the other wave's PE
matmuls.

Per wave, all tensors use a stacked [128, 128] layout: partition p = 32*q + b
(q = hidden quarter 0..3, b = batch row 0..31), free m = hidden-within-quarter.

Gates per step land in ONE PSUM bank [128, 512] = {I|F|G|O} (128 free each),
produced by 4-way col-tiled matmuls (stationary hT chunk [128, 32] bf16 ->
array cols 32q..32q+32, moving W quarter [128, 128] bf16, N=128 @ 1 cyc/row).

Reformulation: state is h~ = 2h; sigma(o) = (tanh(o/2)+1)/2 so the {G|O} bank
gets ONE tanh. Host pre-scales: W_hh *= 0.5 (h~), o-rows of W_hh/W_ih/bias
*= 0.5 (tanh trick), fc1_w *= 0.5 (consumes h~).

Tail per wave-step: ACT sigmoid{I|F}, ACT tanh{G|O}, Pool IG/FC muls,
DVE c = IG+FC, ACT tanh(c), DVE h~ = (TO+1)*TC, PE 4x row-tiled transpose,
DVE copy -> bf16 hT.

PE queue order per step: [w0 transp(t-1), w0 h-MM(t), w0 x(t+1)],
[w1 transp(t-1), w1 h-MM(t), w1 x(t+1)], ... so w1's matmuls run while w0's
tail computes (and vice versa); the only PE stall is transp waiting on the
same wave's tail, hidden behind the other wave's matmul stream.
"""

import sys
from contextlib import ExitStack

if "/opt/trn_rl_repo" not in sys.path:
    sys.path.insert(0, "/opt/trn_rl_repo")

import numpy as np
import ml_dtypes

import concourse.bass as bass
import concourse.tile as tile
from concourse import bacc, mybir
from concourse.bass_utils import run_bass_kernel_spmd
from concourse.masks import make_identity

F32 = mybir.dt.float32
BF16 = mybir.dt.bfloat16
AF = mybir.ActivationFunctionType
ALU = mybir.AluOpType
NPBF = ml_dtypes.bfloat16

B, T, H, HALF, TGT = 512, 256, 512, 256, 28
NCORES = 8
BL = B // NCORES          # 64 batch rows per core
WB = 32                   # batch rows per wave
NW = 2                    # waves per core
KCH = 4                   # contraction chunks of 128 (H=512)
NQ = 4                    # hidden quarters of 128

_cached = {}


def build_program(reps=1, T_override=None, skip_fc=False, skip_tail=False):
    Tl = T_override or T
    nc = bacc.Bacc("TRN2", target_bir_lowering=False, debug=False,
                   num_devices=NCORES)

    # whh packed (gate X, k, q) -> [128, 128] transposed chunk
    d_whh = nc.dram_tensor("whh", [128, 4 * KCH * NQ * 128], BF16,
                           kind="ExternalInput")
    # wxb packed (gate X, q): row0 = W_ih slice, row1 = bias slice
    d_wxb = nc.dram_tensor("wxb", [2, 4 * NQ * 128], BF16, kind="ExternalInput")
    # sequence per wave: [2, NW * T * WB] (row1 = ones), wave-major, t-major
    d_seq = nc.dram_tensor("seqones", [2, NW * T * WB], BF16,
                           kind="ExternalInput")
    d_f1w = nc.dram_tensor("fc1w", [128, KCH * HALF], BF16, kind="ExternalInput")
    d_f1b = nc.dram_tensor("fc1b", [1, HALF], BF16, kind="ExternalInput")
    d_f2w = nc.dram_tensor("fc2w", [128, 2 * TGT], BF16, kind="ExternalInput")
    d_f2b = nc.dram_tensor("fc2b", [1, TGT], BF16, kind="ExternalInput")
    d_out = nc.dram_tensor("out", [BL, TGT], F32, kind="ExternalOutput")

    def woff2(k, q):
        return (k * NQ + q) * 512

    with tile.TileContext(nc) as tc, ExitStack() as ctx:
        consts = ctx.enter_context(tc.tile_pool(name="consts", bufs=1))
        seqp = ctx.enter_context(tc.tile_pool(name="seqp", bufs=2))
        acts = ctx.enter_context(tc.tile_pool(name="acts", bufs=3))
        state = ctx.enter_context(tc.tile_pool(name="state", bufs=2))
        pg = ctx.enter_context(tc.tile_pool(name="pg", bufs=2, space="PSUM"))

        sb_whh = consts.tile([128, 4 * KCH * NQ * 128], BF16)
        sb_wxb = consts.tile([2, 4 * NQ * 128], BF16)
        sb_f1w = consts.tile([128, KCH * HALF], BF16)
        sb_f1b = consts.tile([1, HALF], BF16)
        sb_f2w = consts.tile([128, 2 * TGT], BF16)
        sb_f2b = consts.tile([1, TGT], BF16)
        ident = consts.tile([128, 128], F32)
        ones128 = consts.tile([128, 128], F32)
        half128 = consts.tile([128, 128], F32)
        nc.gpsimd.memset(ones128[:], 1.0)
        nc.gpsimd.memset(half128[:], 0.5)
        sb_ones_f = consts.tile([1, WB], F32)
        sb_ones = consts.tile([1, WB], BF16)
        nc.gpsimd.memset(sb_ones_f[:], 1.0)
        nc.vector.tensor_copy(sb_ones[:], sb_ones_f[:])
        for dst, dsrc in ((sb_whh, d_whh), (sb_wxb, d_wxb), (sb_f1w, d_f1w),
                          (sb_f1b, d_f1b), (sb_f2w, d_f2w), (sb_f2b, d_f2b)):
            nc.sync.dma_start(dst[:], dsrc.ap())
        make_identity(nc, ident[:])

        TCH = 64   # seq chunk in steps

        for _rep in range(reps):
            # per-wave python state
            sb_seq = [None, None]
            banks = [None, None]      # current-step PSUM bank per wave
            cst = [None, None]        # S = 2c state per wave
            hst = [None, None]        # halfS = c per wave
            hT = [None, None]         # bf16 stationary per wave
            hprev = [None, None]      # h~ awaiting transpose

            def emit_xmm(w, t):
                """x-part matmuls for wave w step t into a fresh bank."""
                if t % TCH == 0:
                    sb_seq[w] = seqp.tile([2, TCH * WB], BF16, name=f"sq{w}", tag=f"sq{w}")
                    base = w * T * WB + t * WB  # full-T layout
                    nc.sync.dma_start(sb_seq[w][:],
                                      d_seq.ap()[:, base:base + TCH * WB])
                tt = t % TCH
                xs = sb_seq[w][:][:, tt * WB:(tt + 1) * WB]
                bank = pg.tile([128, 512], F32, name=f"bank{w}", tag=f"bank{w}")
                for q in range(NQ):
                    nc.tensor.matmul(
                        bank[:][32 * q:32 * (q + 1), 0:512],
                        xs,
                        sb_wxb[:][:, q * 512:(q + 1) * 512],
                        start=True, stop=(t == 0),
                        tile_position=(0, 32 * q), skip_group_check=True)
                return bank

            def emit_transp(w):
                """Transpose h~(prev step) of wave w into fresh bf16 hT."""
                pT = pg.tile([128, 128], F32, name=f"pT{w}", tag=f"pT{w}", bufs=1)
                for k in range(2):
                    nc.tensor.transpose(
                        pT[:][:, 64 * k:64 * (k + 1)],
                        hprev[w][:][:, 128 * k:128 * (k + 1)],
                        ident[:][0:64, 0:64])
                hT_new = state.tile([128, 128], BF16, name=f"hT{w}", tag=f"hT{w}")
                nc.scalar.copy(hT_new[:], pT[:])
                hT[w] = hT_new

            for w in range(NW):
                banks[w] = emit_xmm(w, 0)

            for t in range(Tl):
                for w in range(NW):
                    bank = banks[w]
                    if t > 0:
                        emit_transp(w)
                        # h-part: one full-bank-row matmul per (k, quarter):
                        # moving = [W_I|W_F|W_G|W_O] cols of quarter q, N=512
                        for k in range(KCH):
                            hk = hT[w][:][:, 32 * k:32 * (k + 1)]
                            for q in range(NQ):
                                nc.tensor.matmul(
                                    bank[:][32 * q:32 * (q + 1), 0:512],
                                    hk,
                                    sb_whh[:][:, woff2(k, q):woff2(k, q) + 512],
                                    start=False, stop=(k == KCH - 1),
                                    tile_position=(0, 32 * q),
                                    skip_group_check=True)
                    # prefetch next step's x-part (PE filler during tail)
                    if t + 1 < Tl:
                        banks[w] = emit_xmm(w, t + 1)

                    # ---- tail: ONE tanh over the whole gate bank.
                    # I,F,O columns are pre-halved in the weights, so
                    # T_X = tanh(x/2) for those gates and tanh(g) for G.
                    # State: S = 2c (cst) plus halfS = c (hst).
                    #   FC' = (T_F+1) * halfS_prev   (= 2 sig(f) c)
                    #   IG' = (T_I+1) * T_G          (= 2 sig(i) tanh(g))
                    #   S   = FC' + IG'              (= 2c)
                    #   TC  = tanh(0.5*S) = tanh(c);  h~ = (T_O+1)*TC = 2h
                    T4 = acts.tile([128, 512], F32, tag=f"T4{w}")
                    TI1 = acts.tile([128, 128], F32, tag=f"TI1{w}")
                    TO1 = acts.tile([128, 128], F32, tag=f"TO1{w}")
                    IGt = acts.tile([128, 128], F32, tag=f"IG{w}")
                    FCt = acts.tile([128, 128], F32, tag=f"FC{w}")
                    TCt = acts.tile([128, 128], F32, tag=f"TC{w}")
                    c_new = state.tile([128, 128], F32, tag=f"c{w}")
                    ch_new = state.tile([128, 128], F32, tag=f"ch{w}")
                    h_new = state.tile([64, 256], F32, tag=f"h{w}")

                    nc.scalar.activation(T4[:], bank[:], AF.Tanh)
                    nc.gpsimd.tensor_add(TI1[:], T4[:][:, 0:128], ones128[:])
                    nc.gpsimd.tensor_mul(IGt[:], TI1[:], T4[:][:, 256:384])
                    nc.gpsimd.tensor_add(TO1[:], T4[:][:, 384:512],
                                         ones128[:])
                    if t > 0:
                        nc.vector.scalar_tensor_tensor(
                            FCt[:], T4[:][:, 128:256], 1.0, hst[w][:],
                            ALU.add, ALU.mult)
                        nc.vector.tensor_add(c_new[:], FCt[:], IGt[:])
                    else:
                        nc.vector.tensor_copy(c_new[:], IGt[:])
                    nc.scalar.activation(TCt[:], c_new[:], AF.Tanh, scale=0.5)
                    # halfS for next step (off the critical chain)
                    nc.vector.tensor_scalar_mul(ch_new[:], c_new[:], 0.5)
                    # h~ halves land on partitions 0:64 (transpose needs base 0)
                    nc.vector.tensor_mul(h_new[:][:, 0:128], TO1[:][0:64, :],
                                         TCt[:][0:64, :])
                    nc.gpsimd.tensor_mul(h_new[:][:, 128:256],
                                         TO1[:][64:128, :],
                                         TCt[:][64:128, :])
                    hst[w] = ch_new
                    cst[w] = c_new
                    hprev[w] = h_new

            # ---- FC head per wave (transpose final h~ first)
            if skip_fc:
                for w in range(NW):
                    res0 = acts.tile([WB, TGT], F32, name=f"res0{w}", tag=f"res{w}")
                    nc.vector.tensor_copy(res0[:], hprev[w][:][0:WB, 0:TGT])
                    nc.sync.dma_start(d_out.ap()[w * WB:(w + 1) * WB, :], res0[:])
                continue
            ones = sb_ones[:]
            for w in range(NW):
                emit_transp(w)
                p_hid = pg.tile([WB, HALF], F32, tag=f"bank{w}")
                nc.tensor.matmul(p_hid[:], ones, sb_f1b[:],
                                 start=True, stop=False)
                for k in range(KCH):
                    nc.tensor.matmul(p_hid[:], hT[w][:][:, 32 * k:32 * (k + 1)],
                                     sb_f1w[:][:, k * HALF:(k + 1) * HALF],
                                     start=False, stop=(k == KCH - 1))
                hid = acts.tile([WB, HALF], F32, tag=f"hid{w}")
                nc.scalar.activation(hid[:], p_hid[:], AF.Relu)

                pTh = pg.tile([128, 2 * WB], F32, tag=f"pT{w}", bufs=1)
                for k in range(2):
                    nc.tensor.transpose(pTh[:][:, k * WB:(k + 1) * WB],
                                        hid[:][:, k * 128:(k + 1) * 128],
                                        ident[:][0:WB, 0:WB])
                hidT = acts.tile([128, 2 * WB], BF16, tag=f"hidT{w}")
                nc.vector.tensor_copy(hidT[:], pTh[:])

                p_out = pg.tile([WB, TGT], F32, tag=f"po{w}", bufs=1)
                nc.tensor.matmul(p_out[:], ones, sb_f2b[:],
                                 start=True, stop=False)
                for k in range(2):
                    nc.tensor.matmul(p_out[:], hidT[:][:, k * WB:(k + 1) * WB],
                                     sb_f2w[:][:, k * TGT:(k + 1) * TGT],
                                     start=False, stop=(k == 1))
                res = acts.tile([WB, TGT], F32, tag=f"res{w}")
                nc.vector.tensor_copy(res[:], p_out[:])
                nc.sync.dma_start(d_out.ap()[w * WB:(w + 1) * WB, :], res[:])

    if not nc.is_finalized():
        nc.finalize()
    return nc


def _prep_shared(W_ih, W_hh, b_ih, b_hh, fc1_w, fc1_b, fc2_w, fc2_b):
    Wh = np.asarray(W_hh, np.float32).copy() * 0.5          # h~ = 2h
    Wi = np.asarray(W_ih, np.float32).copy()
    bs = np.asarray(b_ih + b_hh, np.float32).copy()
    for g in (0, 1, 3):           # i, f, o: sigma(x) = (tanh(x/2)+1)/2
        Wh[g * H:(g + 1) * H] *= 0.5
        Wi[g * H:(g + 1) * H] *= 0.5
        bs[g * H:(g + 1) * H] *= 0.5

    whh = np.empty((128, 4 * KCH * NQ * 128), np.float32)
    wxb = np.empty((2, 4 * NQ * 128), np.float32)
    for k in range(KCH):
        for q in range(NQ):
            off = (k * NQ + q) * 512
            for x in range(4):
                rows = Wh[x * H + 128 * q: x * H + 128 * (q + 1),
                          128 * k:128 * (k + 1)]
                whh[:, off + 128 * x:off + 128 * (x + 1)] = rows.T
    for q in range(NQ):
        for x in range(4):
            off = q * 512 + x * 128
            wxb[0, off:off + 128] = Wi[x * H + 128 * q: x * H + 128 * (q + 1), 0]
            wxb[1, off:off + 128] = bs[x * H + 128 * q: x * H + 128 * (q + 1)]

    f1 = np.asarray(fc1_w, np.float32) * 0.5                # consumes h~
    f1w = np.empty((128, KCH * HALF), np.float32)
    for k in range(KCH):
        f1w[:, k * HALF:(k + 1) * HALF] = f1[:, 128 * k:128 * (k + 1)].T
    f2w = np.empty((128, 2 * TGT), np.float32)
    for k in range(2):
        f2w[:, k * TGT:(k + 1) * TGT] = \
            np.asarray(fc2_w, np.float32)[:, 128 * k:128 * (k + 1)].T
    return {
        "whh": whh.astype(NPBF), "wxb": wxb.astype(NPBF),
        "fc1w": f1w.astype(NPBF),
        "fc1b": np.asarray(fc1_b, np.float32).reshape(1, HALF).astype(NPBF),
        "fc2w": f2w.astype(NPBF),
        "fc2b": np.asarray(fc2_b, np.float32).reshape(1, TGT).astype(NPBF),
    }


def make_in_maps(inputs):
    shared = _prep_shared(
        inputs["W_ih"], inputs["W_hh"], inputs["b_ih"], inputs["b_hh"],
        inputs["fc1_w"], inputs["fc1_b"], inputs["fc2_w"], inputs["fc2_b"])
    seq = np.asarray(inputs["sequence"], np.float32)[:, :, 0]   # [B, T]
    in_maps = []
    for cid in range(NCORES):
        rows = []
        for w in range(NW):
            blk = seq[cid * BL + w * WB: cid * BL + (w + 1) * WB, :]
            rows.append(blk.T.reshape(1, T * WB))                # t-major
        xs = np.concatenate(rows, axis=1)                        # [1, NW*T*WB]
        seqones = np.concatenate([xs, np.ones_like(xs)], axis=0)
        in_maps.append({"seqones": seqones.astype(NPBF), **shared})
    return in_maps


def run(inputs, trace=False, reps=1):
    if reps not in _cached:
        _cached[reps] = build_program(reps)
    nc = _cached[reps]
    in_maps = make_in_maps(inputs)
    br = run_bass_kernel_spmd(nc, in_maps, list(range(NCORES)), trace=trace)
    out = np.concatenate([br.results[i]["out"] for i in range(NCORES)], axis=0)
    return out[:, :, None].astype(np.float32), br


def kernel(**inputs):
    out, _ = run(inputs)
    return out

